# revision 1
# baseline (speedup 1.0000x reference)
"""Trainium2 Bass kernel for ContactDiffusion GNN message passing.

out = latent + K_norm @ msg,  K = (D+eps)^(-alpha_ij) * exp(-D/12), row-normalized,
msg = MLP(latent).

Strategy (8 NeuronCores, SPMD, full inputs in / full output out):
 - Host: KD-sort points spatially; each core owns 1024 contiguous sorted rows.
 - Device per core: pairwise d2 for its [8192 x 1024] K^T slab via a Gram-form
   fp16-split feature matmul (k=18), elementwise K chain on ScalarE
   (ln / exp, single activation-table set), contraction + row-sums on PE.
 - The core's own diagonal block is computed exactly (ACT Square with
   per-partition bias = direct (ci-cj)^2) with exact ln(D+eps); the Gram pass
   suppresses that block via a rank-1 indicator feature.
 - Cross-core close pairs ("stragglers", d2 < 0.09) are deterministically
   suppressed on device via a second rank-1 indicator feature and their exact
   contribution is added back on host using the exported row sums.
 - MLP is sharded (each core computes msg for its rows); msg is AllGathered.
"""

import math
import os
import sys
from contextlib import ExitStack

import numpy as np

sys.path.insert(0, "/opt/trn_rl_repo")

import ml_dtypes

import concourse.bass as bass
import concourse.tile as tile
from concourse import bacc, mybir
from concourse.bass_utils import run_bass_kernel_spmd

F32 = mybir.dt.float32
F16 = mybir.dt.float16
BF16 = mybir.dt.bfloat16
AF = mybir.ActivationFunctionType
ALU = mybir.AluOpType

NP_BF16 = ml_dtypes.bfloat16

N, DIM, NCORE = 8192, 512, 8
NSH = N // NCORE            # rows per core
EPS, LAM = 1e-4, 12.0
TSTRAG = 0.09               # d2 below this across cores -> straggler
SUP = 1e3                   # suppressor feature magnitude (SUP^2 added to d2)
GROUP = 16                  # j-tiles per psum_out accumulation group
LN12 = math.log(12.0)

_BUILT = {}


# ----------------------------------------------------------------------------
# device program
# ----------------------------------------------------------------------------
def build_program(n=N, dim=DIM, nsh=NSH, group=GROUP, trace_sim=False, gelu=True, taps=False):
    nt_own = nsh // 128          # own-block j-tiles
    nt_main = n // 128           # main-pass j-tiles
    n_kd = dim // 128            # contraction k-blocks for MLP
    n_ic = nsh // 128            # i-chunks
    nt_all = nt_own + nt_main

    nc = bacc.Bacc("TRN2", target_bir_lowering=False, debug=False,
                   num_devices=NCORE)

    # ---- dram params ----
    featj = nc.dram_tensor("featj", [18, n], F16, kind="ExternalInput").ap()
    feati = nc.dram_tensor("feati", [18, nsh], F16, kind="ExternalInput").ap()
    ahj = nc.dram_tensor("ahj", [128, nt_main], F32, kind="ExternalInput").ap()
    ahjo = nc.dram_tensor("ahjo", [128, nt_own], F32, kind="ExternalInput").ap()
    ahibc = nc.dram_tensor("ahibc", [128, nsh], F32, kind="ExternalInput").ap()
    cib = nc.dram_tensor("cib", [128, 3 * nsh], F32, kind="ExternalInput").ap()
    ncjo = nc.dram_tensor("ncjo", [128, 3 * nt_own], F32, kind="ExternalInput").ap()
    latT = nc.dram_tensor("latT", [dim, nsh], F16, kind="ExternalInput").ap()
    w1t = nc.dram_tensor("w1t", [dim, dim], F16, kind="ExternalInput").ap()
    w2t = nc.dram_tensor("w2t", [dim, dim], F16, kind="ExternalInput").ap()
    b1c = nc.dram_tensor("b1c", [128, n_kd], F32, kind="ExternalInput").ap()
    b2r = nc.dram_tensor("b2r", [1, dim], F16, kind="ExternalInput").ap()
    onescol = nc.dram_tensor("onescol", [1, 128], F16, kind="ExternalInput").ap()
    ones128 = nc.dram_tensor("ones128", [128, 1], BF16, kind="ExternalInput").ap()

    num_out = nc.dram_tensor("num", [nsh, dim], F32, kind="ExternalOutput").ap()
    tap_aps = {}
    if taps:
        for tn in ["tap_d2", "tap_l", "tap_d12", "tap_t", "tap_k", "tap_kown"]:
            tap_aps[tn] = nc.dram_tensor(tn, [128, nsh], F32, kind="ExternalOutput").ap()
    srow_out = nc.dram_tensor("srow", [128, n_ic], F32, kind="ExternalOutput").ap()

    with tile.TileContext(nc, trace_sim=trace_sim) as tc, ExitStack() as ctx:
        pers = ctx.enter_context(tc.tile_pool(name="pers", bufs=1))
        p_big = ctx.enter_context(tc.tile_pool(name="pbig", bufs=2, space="PSUM"))
        p_out = ctx.enter_context(tc.tile_pool(name="pout", bufs=2, space="PSUM"))
        p_s = ctx.enter_context(tc.tile_pool(name="ps", bufs=1, space="PSUM"))
        sq_pool = ctx.enter_context(tc.tile_pool(name="sq", bufs=1))
        l_pool = ctx.enter_context(tc.tile_pool(name="lp", bufs=2))
        d12_pool = ctx.enter_context(tc.tile_pool(name="d12", bufs=2))
        amt_pool = ctx.enter_context(tc.tile_pool(name="amt", bufs=2))
        k_pool = ctx.enter_context(tc.tile_pool(name="kp", bufs=group + 4))
        kraw_pool = ctx.enter_context(tc.tile_pool(name="kraw", bufs=2))
        msg_pool = ctx.enter_context(tc.tile_pool(name="msgp", bufs=group + 4))
        dram = ctx.enter_context(tc.tile_pool(name="dram", bufs=1, space="DRAM"))
        tapp = ctx.enter_context(tc.tile_pool(name="tapp", bufs=2)) if taps else None

        dma = nc.sync.dma_start

        # ---- persistent SBUF loads ----
        featj_sb = pers.tile([18, n], F16)
        dma(featj_sb[:], featj[:])
        feati_sb = pers.tile([18, nsh], F16)
        dma(feati_sb[:], feati[:])
        ahj_sb = pers.tile([128, nt_main], F32)
        dma(ahj_sb[:], ahj[:])
        ahjo_sb = pers.tile([128, nt_own], F32)
        dma(ahjo_sb[:], ahjo[:])
        ahibc_sb = pers.tile([128, nsh], F32)
        dma(ahibc_sb[:], ahibc[:])
        cib_sb = pers.tile([128, 3 * nsh], F32)
        dma(cib_sb[:], cib[:])
        ncjo_sb = pers.tile([128, 3 * nt_own], F32)
        dma(ncjo_sb[:], ncjo[:])
        b1c_sb = pers.tile([128, n_kd], F32)
        dma(b1c_sb[:], b1c[:])
        b2r_sb = pers.tile([1, dim], F16)
        dma(b2r_sb[:], b2r[:])
        onescol_sb = pers.tile([1, 128], F16)
        dma(onescol_sb[:], onescol[:])
        ones128_sb = pers.tile([128, 1], BF16)
        dma(ones128_sb[:], ones128[:])
        latT_sb = [pers.tile([128, nsh], F16, tag=f"latT{k}", name=f"latT{k}") for k in range(n_kd)]
        for k in range(n_kd):
            dma(latT_sb[k][:], latT[k * 128:(k + 1) * 128, :])
        w1t_sb = [pers.tile([128, dim], F16, tag=f"w1t{k}", name=f"w1t{k}") for k in range(n_kd)]
        w2t_sb = [pers.tile([128, dim], F16, tag=f"w2t{k}", name=f"w2t{k}") for k in range(n_kd)]
        for k in range(n_kd):
            dma(w1t_sb[k][:], w1t[k * 128:(k + 1) * 128, :])
            dma(w2t_sb[k][:], w2t[k * 128:(k + 1) * 128, :])

        acc = pers.tile([128, n_ic * dim], F32)       # out accumulators
        nc.vector.memset(acc[:], 0.0)

        bias_ln12 = pers.tile([128, 1], F32)
        nc.gpsimd.memset(bias_ln12[:], -LN12)
        bias_eps = pers.tile([128, 1], F32)
        nc.gpsimd.memset(bias_eps[:], EPS)
        bias_ln6 = pers.tile([128, 1], F32)
        nc.gpsimd.memset(bias_ln6[:], -math.log(6.0))

        msgown_d = dram.tile([nsh, dim], BF16)
        msgall_d = dram.tile([n, dim], BF16)

        # ---- phase A: MLP (gelu table set) ----
        cw = min(512, nsh)
        hT_sb = [pers.tile([128, nsh], F16, tag=f"hT{k}", name=f"hT{k}") for k in range(n_kd)]
        for mc in range(n_kd):
            ph = p_big.tile([128, nsh], F32, tag="big", name="ph")
            for half in range(nsh // cw):
                hs = slice(half * cw, (half + 1) * cw)
                for kb in range(n_kd):
                    nc.tensor.matmul(
                        ph[:, hs],
                        lhsT=w1t_sb[kb][:, mc * 128:(mc + 1) * 128],
                        rhs=latT_sb[kb][:, hs],
                        start=(kb == 0), stop=(kb == n_kd - 1))
            nc.scalar.activation(hT_sb[mc][:], ph[:], AF.Gelu if gelu else AF.Identity,
                                 bias=b1c_sb[:, mc:mc + 1], scale=1.0)

        msgown_sb = [pers.tile([128, dim], BF16, tag=f"mo{ic}", name=f"mo{ic}") for ic in range(n_ic)]
        for ic in range(n_ic):
            pm = p_out.tile([128, dim], F32, tag="out", name="pm")
            for kb in range(n_kd):
                nc.tensor.matmul(
                    pm[:],
                    lhsT=hT_sb[kb][:, ic * 128:(ic + 1) * 128],
                    rhs=w2t_sb[kb][:],
                    start=(kb == 0), stop=False)
            nc.tensor.matmul(pm[:], lhsT=onescol_sb[:], rhs=b2r_sb[:],
                             start=False, stop=True)
            nc.scalar.copy(msgown_sb[ic][:], pm[:])
            dma(msgown_d[ic * 128:(ic + 1) * 128, :], msgown_sb[ic][:])

        # ---- phase B: AllGather msg ----
        nc.gpsimd.collective_compute(
            "AllGather", ALU.bypass,
            ins=[msgown_d.opt()], outs=[msgall_d.opt()],
            replica_groups=[list(range(NCORE))])

        # ---- phase C/D: slab loop ----
        ps_s = p_s.tile([128, n_ic], F32)

        def emit_elementwise(jt):
            """produce K tile [128, nsh] bf16 + its msg rhs tile; return both"""
            if jt < nt_own:
                # own-block exact pass
                t = jt
                sqs = []
                for d in range(3):
                    sq = sq_pool.tile([128, nsh], F32, tag=f"sq{d}")
                    nc.scalar.activation(
                        sq[:], cib_sb[:, d * nsh:(d + 1) * nsh], AF.Square,
                        bias=ncjo_sb[:, (t * 3 + d):(t * 3 + d) + 1], scale=1.0)
                    sqs.append(sq)
                nc.vector.tensor_tensor(sqs[0][:], sqs[0][:], sqs[1][:], op=ALU.add)
                nc.vector.tensor_tensor(sqs[0][:], sqs[0][:], sqs[2][:], op=ALU.add)
                l = l_pool.tile([128, nsh], F32)
                nc.scalar.activation(l[:], sqs[0][:], AF.Ln)
                d12 = d12_pool.tile([128, nsh], F32)
                nc.scalar.activation(d12[:], l[:], AF.Exp, bias=bias_ln12[:, 0:1], scale=0.5)
                bigL = amt_pool.tile([128, nsh], F32, tag="bigL")
                nc.scalar.activation(bigL[:], d12[:], AF.Ln, bias=bias_eps[:, 0:1], scale=12.0)
                al = amt_pool.tile([128, nsh], F32, tag="alpha")
                nc.vector.tensor_scalar_add(al[:], ahibc_sb[:], ahjo_sb[:, t:t + 1])
                m = amt_pool.tile([128, nsh], F32, tag="m")
                nc.vector.tensor_tensor(m[:], al[:], bigL[:], op=ALU.mult)
                tt = amt_pool.tile([128, nsh], F32, tag="t")
                nc.gpsimd.tensor_tensor(tt[:], m[:], d12[:], op=ALU.add)
                kraw = kraw_pool.tile([128, nsh], BF16)
                nc.scalar.activation(kraw[:], tt[:], AF.Exp, scale=-1.0)
                ktile = k_pool.tile([128, nsh], BF16)
                nc.gpsimd.affine_select(
                    ktile[:], kraw[:], pattern=[[1, nsh]],
                    compare_op=ALU.not_equal, fill=0.0,
                    base=-(t * 128), channel_multiplier=-1)
                if taps and t == 0:
                    tapk = tapp.tile([128, nsh], F32, tag="tap", name="tapkown")
                    nc.scalar.copy(tapk[:], ktile[:])
                    dma(tap_aps["tap_kown"][:], tapk[:])
                return ktile, msgown_sb[t]
            # main pass (gram)
            t = jt - nt_own
            pd2 = p_big.tile([128, nsh], F32, tag="big", name="pd2")
            for half in range(nsh // cw):
                hs = slice(half * cw, (half + 1) * cw)
                nc.tensor.matmul(pd2[:, hs],
                                 lhsT=featj_sb[:, t * 128:(t + 1) * 128],
                                 rhs=feati_sb[:, hs],
                                 start=True, stop=True)
            l = l_pool.tile([128, nsh], F32)
            nc.scalar.activation(l[:], pd2[:], AF.Ln)
            d12 = d12_pool.tile([128, nsh], F32)
            nc.scalar.activation(d12[:], l[:], AF.Exp, bias=bias_ln6[:, 0:1], scale=0.5)
            al = amt_pool.tile([128, nsh], F32, tag="alpha")
            nc.vector.tensor_scalar_add(al[:], ahibc_sb[:], ahj_sb[:, t:t + 1])
            m = amt_pool.tile([128, nsh], F32, tag="m")
            nc.vector.tensor_tensor(m[:], al[:], l[:], op=ALU.mult)
            tt = amt_pool.tile([128, nsh], F32, tag="t")
            nc.gpsimd.tensor_tensor(tt[:], m[:], d12[:], op=ALU.add)
            ktile = k_pool.tile([128, nsh], BF16)
            nc.scalar.activation(ktile[:], tt[:], AF.Exp, scale=-0.5)
            if taps and t == 8:
                for nm, src in [("tap_d2", pd2), ("tap_l", l), ("tap_d12", d12), ("tap_t", tt)]:
                    tp = tapp.tile([128, nsh], F32, tag="tap", name=f"tp{nm}")
                    nc.scalar.copy(tp[:], src[:])
                    dma(tap_aps[nm][:], tp[:])
                tpk = tapp.tile([128, nsh], F32, tag="tap", name="tpk")
                nc.scalar.copy(tpk[:], ktile[:])
                dma(tap_aps["tap_k"][:], tpk[:])
            mt = msg_pool.tile([128, dim], BF16)
            dma(mt[:], msgall_d[t * 128:(t + 1) * 128, :])
            return ktile, mt

        jt = 0
        while jt < nt_all:
            g = min(group, nt_all - jt)
            tiles = [emit_elementwise(jt + i) for i in range(g)]
            # row-sum matmuls (persistent psum_s accumulation)
            for i, (kt, _) in enumerate(tiles):
                for ic in range(n_ic):
                    nc.tensor.matmul(
                        ps_s[:, ic:ic + 1],
                        lhsT=kt[:, ic * 128:(ic + 1) * 128],
                        rhs=ones128_sb[:],
                        start=(jt + i == 0 and ic == 0),
                        stop=(jt + i == nt_all - 1))
            # contraction for this group
            for ic in range(n_ic):
                po = p_out.tile([128, dim], F32, tag="out", name="po")
                for i, (kt, mt) in enumerate(tiles):
                    nc.tensor.matmul(
                        po[:],
                        lhsT=kt[:, ic * 128:(ic + 1) * 128],
                        rhs=mt[:],
                        start=(i == 0), stop=(i == g - 1))
                asl = slice(ic * dim, (ic + 1) * dim)
                nc.vector.tensor_tensor(acc[:, asl], acc[:, asl], po[:], op=ALU.add)
            jt += g

        # ---- epilogue ----
        ssb = pers.tile([128, n_ic], F32)
        nc.scalar.copy(ssb[:], ps_s[:])
        dma(srow_out[:], ssb[:])
        for ic in range(n_ic):
            dma(num_out[ic * 128:(ic + 1) * 128, :],
                acc[:, ic * dim:(ic + 1) * dim])

    nc.compile()
    return nc


# ----------------------------------------------------------------------------
# host-side preprocessing
# ----------------------------------------------------------------------------
def _kdsort(coords, nblocks):
    def rec(idx, nb):
        if nb == 1:
            return [idx]
        pts = coords[idx]
        ax = int(np.argmax(pts.max(0) - pts.min(0)))
        order = np.argsort(pts[:, ax], kind="stable")
        half = len(idx) // 2
        return rec(idx[order[:half]], nb // 2) + rec(idx[order[half:]], nb // 2)

    return np.concatenate(rec(np.arange(coords.shape[0]), nblocks))


def _split16(x):
    hi = x.astype(np.float16).astype(np.float32)
    lo = (x - hi).astype(np.float16).astype(np.float32)
    return hi, lo


_erf = np.vectorize(math.erf)


def kernel(latent, coords, alpha, W1, b1, W2, b2):
    latent = np.asarray(latent, np.float32)
    coords = np.asarray(coords, np.float32)
    alpha = np.asarray(alpha, np.float32)
    W1 = np.asarray(W1, np.float32)
    b1 = np.asarray(b1, np.float32)
    W2 = np.asarray(W2, np.float32)
    b2 = np.asarray(b2, np.float32)

    perm = _kdsort(coords.astype(np.float64), 64)
    cs = coords[perm]
    als = alpha[perm]
    lats = latent[perm]
    c64 = cs.astype(np.float64)

    core_of = np.arange(N) // NSH
    # stragglers: cross-core pairs with d2 < TSTRAG
    Jstar = [set() for _ in range(NCORE)]
    Istar = [set() for _ in range(NCORE)]
    for i0 in range(0, N, 1024):
        blk = cs[i0:i0 + 1024].astype(np.float64)
        d2b = ((blk[:, None, :] - c64[None, :, :]) ** 2).sum(-1)
        d2b[np.arange(1024), np.arange(i0, i0 + 1024)] = np.inf
        ii, jj = np.nonzero(d2b < TSTRAG)
        ii = ii + i0
        msk = core_of[ii] != core_of[jj]
        for a, b in zip(ii[msk], jj[msk]):
            c = core_of[a]
            Jstar[c].add(int(b))
            Istar[c].add(int(a - c * NSH))

    r = (c64 ** 2).sum(-1).astype(np.float32)
    a2 = (-2.0 * cs).astype(np.float32)
    chj = [_split16(cs[:, d]) for d in range(3)]
    ahi = [_split16(a2[:, d]) for d in range(3)]
    rj = _split16(r)

    in_maps = []
    for core in range(NCORE):
        blk = slice(core * NSH, (core + 1) * NSH)
        rows_j, rows_i = [], []
        for d in range(3):
            for (jp, ip) in [(chj[d][0], ahi[d][0]), (chj[d][0], ahi[d][1]),
                             (chj[d][1], ahi[d][0]), (chj[d][1], ahi[d][1])]:
                rows_j.append(jp)
                rows_i.append(ip[blk])
        ones = np.ones(N, np.float32)
        onesi = np.ones(NSH, np.float32)
        rows_j += [rj[0], rj[1]]
        rows_i += [onesi, onesi]
        rows_j += [ones, ones]
        rows_i += [rj[0][blk], rj[1][blk]]
        mown = np.zeros(N, np.float32)
        mown[blk] = SUP
        rows_j += [mown]
        rows_i += [np.full(NSH, SUP, np.float32)]
        g = np.zeros(N, np.float32)
        h = np.zeros(NSH, np.float32)
        for j in Jstar[core]:
            g[j] = SUP
        for i in Istar[core]:
            h[i] = SUP
        rows_j += [g]
        rows_i += [h]
        featj = np.stack(rows_j).astype(np.float16)
        feati = np.stack(rows_i).astype(np.float16)

        ah = (als / 2.0).astype(np.float32)
        ahj = ah.reshape(64, 128).T.copy()                      # [128, 64]
        ahjo = ah[blk].reshape(8, 128).T.copy()                 # [128, 8]
        ahibc = np.broadcast_to(ah[blk], (128, NSH)).copy()
        cib = np.concatenate(
            [np.broadcast_to(cs[blk, d], (128, NSH)) for d in range(3)],
            axis=1).astype(np.float32).copy()                   # [128, 3072]
        ncjo = np.empty((128, 24), np.float32)
        for t in range(8):
            for d in range(3):
                ncjo[:, t * 3 + d] = -cs[core * NSH + t * 128:
                                         core * NSH + (t + 1) * 128, d]
        in_maps.append({
            "featj": featj, "feati": feati,
            "ahj": np.ascontiguousarray(ahj),
            "ahjo": np.ascontiguousarray(ahjo),
            "ahibc": ahibc, "cib": cib, "ncjo": ncjo,
            "latT": lats[blk].T.astype(np.float16).copy(),
            "w1t": W1.T.astype(np.float16).copy(),
            "w2t": W2.T.astype(np.float16).copy(),
            "b1c": b1.reshape(4, 128).T.astype(np.float32).copy(),
            "b2r": b2.reshape(1, DIM).astype(np.float16),
            "onescol": np.ones((1, 128), np.float16),
            "ones128": np.ones((128, 1), NP_BF16),
        })

    if "nc" not in _BUILT:
        _BUILT["nc"] = build_program()
    nc = _BUILT["nc"]
    res = run_bass_kernel_spmd(nc, in_maps, core_ids=list(range(NCORE)))

    num_all = np.zeros((N, DIM), np.float32)
    s_all = np.zeros(N, np.float32)
    for core in range(NCORE):
        blk = slice(core * NSH, (core + 1) * NSH)
        num_all[blk] = res.results[core]["num"]
        s_all[blk] = res.results[core]["srow"].T.reshape(-1)

    # host fix: add back exact K for suppressed straggler grid J* x I*
    need_rows = sorted(set().union(*Jstar)) if any(Jstar) else []
    if need_rows:
        lr = lats[need_rows]
        hh = lr @ W1.T + b1
        hh = (hh * 0.5 * (1.0 + _erf(hh / np.sqrt(2.0)))).astype(np.float32)
        msg_rows = (hh @ W2.T + b2).astype(np.float32)
        row_pos = {j: k for k, j in enumerate(need_rows)}
        for core in range(NCORE):
            J = sorted(Jstar[core])
            I = sorted(Istar[core])
            if not J or not I:
                continue
            Ig = np.array(I) + core * NSH
            d2c = ((c64[J][:, None, :] - c64[Ig][None, :, :]) ** 2).sum(-1)
            Dc = np.sqrt(d2c)
            aijc = (als[J].astype(np.float64)[:, None]
                    + als[Ig].astype(np.float64)[None, :]) * 0.5
            Kc = (Dc + EPS) ** (-aijc) * np.exp(-Dc / LAM)
            mrows = msg_rows[[row_pos[j] for j in J]]
            num_all[Ig] += (Kc.T @ mrows).astype(np.float32)
            s_all[Ig] += Kc.sum(0).astype(np.float32)

    out = lats + num_all / (s_all[:, None] + 1e-8)
    final = np.empty_like(out)
    final[perm] = out
    return final.astype(np.float32)



# revision 4
# speedup vs baseline: 1.3523x; 1.3523x over previous
"""Trainium2 Bass kernel for ContactDiffusion GNN message passing.

out = latent + K_norm @ msg,  K = (D+eps)^(-alpha_ij) * exp(-D/12), row-normalized,
msg = MLP(latent).

v2 design (8 NeuronCores, SPMD, full inputs in / full output out):
 - Host: KD-sort points spatially; each core owns 1024 contiguous sorted rows.
 - Per-core j-block order is ROTATED by the core id so every core's own block
   sits at slots 0..7 of the 64-slot slab loop -> identical SPMD program.
 - Device: one unified Gram pass for all 64 j-tiles ([128 j x 1024 i] via a
   17-feature fp16-split matmul), elementwise chain Ln -> Exp(D/6) on ScalarE
   (single activation table set, patched chooser), (ah_i+ah_j)*l and +D/6 on
   DVE, final Exp -> bf16 K with free row-sum accumulation (accum_out).
 - Diagonal of own slots: Ln bias keeps d2>0; affine_select zeroes the diag;
   row sums for those slots via DVE reduce post-select.
 - Close pairs (d2 < TSTRAG, i != j, any core) suppressed on device via a
   rank-1 indicator feature; exact K added back on host from the device's own
   bf16 msg output. Row sums assembled on host by symmetry (column partials).
 - MLP sharded; msg AllGathered in two halves so contraction of half the
   slots can start ~30us earlier; per-slot msg tiles fetched with indirect
   DMA driven by a per-core index table (realizes the rotation).
"""

import math
import sys
from contextlib import ExitStack

import numpy as np

sys.path.insert(0, "/opt/trn_rl_repo")

import ml_dtypes

import concourse.bass as bass
import concourse.tile as tile
from concourse import bacc, mybir
from concourse.bass_utils import run_bass_kernel_spmd

F32 = mybir.dt.float32
F16 = mybir.dt.float16
BF16 = mybir.dt.bfloat16
I32 = mybir.dt.int32
AF = mybir.ActivationFunctionType
ALU = mybir.AluOpType

NP_BF16 = ml_dtypes.bfloat16

N, DIM, NCORE = 8192, 512, 8
NSH = N // NCORE
NT = N // 128               # 64 j-slots
EPS, LAM = 1e-4, 12.0
TSTRAG = 0.09
SUP = 1e3
B_LN = 2e-3                 # Ln input bias: keeps diag d2 > 0 (gram err <1.2e-3)
GROUP = 16
LN6 = math.log(6.0)

_BUILT = {}


def _patch_act_tables():
    """Force the activation-table chooser to keep Ln/Exp/Square/Copy in the
    combined natural_log_exp set (and Gelu alone in its set) so the slab loop
    runs with zero table reloads. Only narrows choices; emitted set ids still
    index the true act_info.json order."""
    import concourse.hw_specs as hw_specs

    if getattr(hw_specs.get_activation_tables, "_patched_v2", False):
        return
    orig = hw_specs.get_activation_tables
    contested = {AF.Ln, AF.Exp, AF.Square, AF.Copy, AF.Identity, AF.Gelu,
                 AF.MemsetZero}

    def patched(arch):
        tabs = orig(arch)
        out = {}
        for name, s in tabs.items():
            if name == "natural_log_exp_and_others":
                out[name] = set(s) | {AF.Copy, AF.Identity}
            elif name == "gelu_and_others":
                out[name] = (set(s) - contested) | {AF.Gelu}
            else:
                out[name] = set(s) - contested
        return out

    patched._patched_v2 = True
    hw_specs.get_activation_tables = patched
    bacc.get_activation_tables = patched


# ----------------------------------------------------------------------------
# device program
# ----------------------------------------------------------------------------
def build_program(trace_sim=False):
    nsh = NSH
    n_kd = DIM // 128           # 4 contraction k-blocks for MLP
    n_ic = nsh // 128           # 8 i-chunks

    _patch_act_tables()
    nc = bacc.Bacc("TRN2", target_bir_lowering=False, debug=False,
                   num_devices=NCORE)

    featj = nc.dram_tensor("featj", [17, N], F16, kind="ExternalInput").ap()
    feati = nc.dram_tensor("feati", [17, nsh], F16, kind="ExternalInput").ap()
    ahj = nc.dram_tensor("ahj", [128, NT], F32, kind="ExternalInput").ap()
    ahibc = nc.dram_tensor("ahibc", [128, nsh], F32, kind="ExternalInput").ap()
    idxt = nc.dram_tensor("idxt", [128, NT], I32, kind="ExternalInput").ap()
    latT = nc.dram_tensor("latT", [DIM, nsh], F16, kind="ExternalInput").ap()
    w1t = nc.dram_tensor("w1t", [DIM, DIM], F16, kind="ExternalInput").ap()
    w2t = nc.dram_tensor("w2t", [DIM, DIM], F16, kind="ExternalInput").ap()
    b1c = nc.dram_tensor("b1c", [128, n_kd], F32, kind="ExternalInput").ap()
    b2r = nc.dram_tensor("b2r", [1, DIM], F16, kind="ExternalInput").ap()
    onescol = nc.dram_tensor("onescol", [1, 128], F16, kind="ExternalInput").ap()

    num_out = nc.dram_tensor("num", [nsh, DIM], F32, kind="ExternalOutput").ap()
    srow_out = nc.dram_tensor("srow", [128, NT], F32, kind="ExternalOutput").ap()
    msgo_out = nc.dram_tensor("msgo", [nsh, DIM], BF16, kind="ExternalOutput").ap()

    with tile.TileContext(nc, trace_sim=trace_sim) as tc, ExitStack() as ctx:
        pers = ctx.enter_context(tc.tile_pool(name="pers", bufs=1))
        p_big = ctx.enter_context(tc.tile_pool(name="pbig", bufs=2, space="PSUM"))
        p_out = ctx.enter_context(tc.tile_pool(name="pout", bufs=2, space="PSUM"))
        l_pool = ctx.enter_context(tc.tile_pool(name="lp", bufs=3))
        d12_pool = ctx.enter_context(tc.tile_pool(name="d12", bufs=3))
        m_pool = ctx.enter_context(tc.tile_pool(name="mp", bufs=2))
        t_pool = ctx.enter_context(tc.tile_pool(name="tp", bufs=2))
        k_pool = ctx.enter_context(tc.tile_pool(name="kp", bufs=30))
        kraw_pool = ctx.enter_context(tc.tile_pool(name="kraw", bufs=2))
        msg_pool = ctx.enter_context(tc.tile_pool(name="msgp", bufs=18))
        dram = ctx.enter_context(tc.tile_pool(name="dram", bufs=1, space="DRAM"))

        dma = nc.sync.dma_start

        # ---- persistent SBUF loads ----
        featj_sb = pers.tile([17, N], F16)
        dma(featj_sb[:], featj[:])
        feati_sb = pers.tile([17, nsh], F16)
        dma(feati_sb[:], feati[:])
        ahj_sb = pers.tile([128, NT], F32)
        dma(ahj_sb[:], ahj[:])
        ahibc_sb = pers.tile([128, nsh], F32)
        dma(ahibc_sb[:], ahibc[:])
        idx_sb = pers.tile([128, NT], I32)
        dma(idx_sb[:], idxt[:])
        b1c_sb = pers.tile([128, n_kd], F32)
        dma(b1c_sb[:], b1c[:])
        b2r_sb = pers.tile([1, DIM], F16)
        dma(b2r_sb[:], b2r[:])
        onescol_sb = pers.tile([1, 128], F16)
        dma(onescol_sb[:], onescol[:])
        latT_sb = [pers.tile([128, nsh], F16, tag=f"latT{k}", name=f"latT{k}") for k in range(n_kd)]
        for k in range(n_kd):
            dma(latT_sb[k][:], latT[k * 128:(k + 1) * 128, :])
        w1t_sb = [pers.tile([128, DIM], F16, tag=f"w1t{k}", name=f"w1t{k}") for k in range(n_kd)]
        w2t_sb = [pers.tile([128, DIM], F16, tag=f"w2t{k}", name=f"w2t{k}") for k in range(n_kd)]
        for k in range(n_kd):
            dma(w1t_sb[k][:], w1t[k * 128:(k + 1) * 128, :])
            dma(w2t_sb[k][:], w2t[k * 128:(k + 1) * 128, :])

        acc = pers.tile([128, n_ic * DIM], F32)
        nc.vector.memset(acc[:], 0.0)
        sacc = pers.tile([128, NT], F32)

        bias_b = pers.tile([128, 1], F32)
        nc.gpsimd.memset(bias_b[:], B_LN)
        bias_ln6 = pers.tile([128, 1], F32)
        nc.gpsimd.memset(bias_ln6[:], -LN6)

        msgown_d = dram.tile([nsh, DIM], BF16)
        msgall1_d = dram.tile([N // 2, DIM], BF16)   # halves of all-gathered msg
        msgall2_d = dram.tile([N // 2, DIM], BF16)

        # ---- phase A: MLP ----
        cw = 512
        hT_sb = [pers.tile([128, nsh], F16, tag=f"hT{k}", name=f"hT{k}") for k in range(n_kd)]
        for mc in range(n_kd):
            ph = p_big.tile([128, nsh], F32, tag="big", name="ph")
            for half in range(nsh // cw):
                hs = slice(half * cw, (half + 1) * cw)
                for kb in range(n_kd):
                    nc.tensor.matmul(
                        ph[:, hs],
                        lhsT=w1t_sb[kb][:, mc * 128:(mc + 1) * 128],
                        rhs=latT_sb[kb][:, hs],
                        start=(kb == 0), stop=(kb == n_kd - 1))
            nc.scalar.activation(hT_sb[mc][:], ph[:], AF.Gelu,
                                 bias=b1c_sb[:, mc:mc + 1], scale=1.0)

        msgown_sb = [pers.tile([128, DIM], BF16, tag=f"mo{ic}", name=f"mo{ic}") for ic in range(n_ic)]

        def emit_msgown(ic):
            pm = p_out.tile([128, DIM], F32, tag="out", name="pm")
            for kb in range(n_kd):
                nc.tensor.matmul(
                    pm[:],
                    lhsT=hT_sb[kb][:, ic * 128:(ic + 1) * 128],
                    rhs=w2t_sb[kb][:],
                    start=(kb == 0), stop=False)
            nc.tensor.matmul(pm[:], lhsT=onescol_sb[:], rhs=b2r_sb[:],
                             start=False, stop=True)
            nc.scalar.copy(msgown_sb[ic][:], pm[:])
            dma(msgown_d[ic * 128:(ic + 1) * 128, :], msgown_sb[ic][:])
            dma(msgo_out[ic * 128:(ic + 1) * 128, :], msgown_sb[ic][:])

        for ic in range(4):
            emit_msgown(ic)
        # AllGather first half of msg rows early
        nc.gpsimd.collective_compute(
            "AllGather", ALU.bypass,
            ins=[msgown_d[0:512, :].opt()], outs=[msgall1_d.opt()],
            replica_groups=[list(range(NCORE))])
        for ic in range(4, 8):
            emit_msgown(ic)
        nc.gpsimd.collective_compute(
            "AllGather", ALU.bypass,
            ins=[msgown_d[512:1024, :].opt()], outs=[msgall2_d.opt()],
            replica_groups=[list(range(NCORE))])

        # ---- slab loop ----
        # slot order: own slots first, then AG1-gated slots (s%8<4), then rest
        order = list(range(8)) \
            + [s for s in range(8, NT) if s % 8 < 4] \
            + [s for s in range(8, NT) if s % 8 >= 4]

        def emit_elementwise(s):
            pd2 = p_big.tile([128, nsh], F32, tag="big", name="pd2")
            for half in range(nsh // cw):
                hs = slice(half * cw, (half + 1) * cw)
                nc.tensor.matmul(pd2[:, hs],
                                 lhsT=featj_sb[:, s * 128:(s + 1) * 128],
                                 rhs=feati_sb[:, hs],
                                 start=True, stop=True)
            l = l_pool.tile([128, nsh], F32)
            nc.scalar.activation(l[:], pd2[:], AF.Ln, bias=bias_b[:, 0:1])
            d12 = d12_pool.tile([128, nsh], BF16)
            nc.scalar.activation(d12[:], l[:], AF.Exp, bias=bias_ln6[:, 0:1],
                                 scale=0.5)
            m = m_pool.tile([128, nsh], F32)
            nc.vector.scalar_tensor_tensor(
                m[:], ahibc_sb[:], ahj_sb[:, s:s + 1], l[:],
                op0=ALU.add, op1=ALU.mult)
            t = t_pool.tile([128, nsh], F32)
            nc.vector.tensor_tensor(t[:], m[:], d12[:], op=ALU.add)
            if s < 8:
                kraw = kraw_pool.tile([128, nsh], BF16, tag="kraw", name="kraw")
                nc.scalar.activation(kraw[:], t[:], AF.Exp, scale=-0.5)
                ktile = k_pool.tile([128, nsh], BF16)
                nc.gpsimd.affine_select(
                    ktile[:], kraw[:], pattern=[[1, nsh]],
                    compare_op=ALU.not_equal, fill=0.0,
                    base=-(s * 128), channel_multiplier=-1)
                nc.vector.tensor_reduce(
                    sacc[:, s:s + 1], ktile[:], axis=mybir.AxisListType.X,
                    op=ALU.add)
                return ktile, msgown_sb[s]
            ktile = k_pool.tile([128, nsh], BF16)
            nc.scalar.activation(ktile[:], t[:], AF.Exp, scale=-0.5,
                                 accum_out=sacc[:, s:s + 1])
            mt = msg_pool.tile([128, DIM], BF16)
            src = msgall1_d if s % 8 < 4 else msgall2_d
            nc.gpsimd.indirect_dma_start(
                out=mt[:], out_offset=None, in_=src[:],
                in_offset=bass.IndirectOffsetOnAxis(ap=idx_sb[:, s:s + 1],
                                                    axis=0))
            return ktile, mt

        pos = 0
        while pos < NT:
            g = min(GROUP, NT - pos)
            tiles = [emit_elementwise(order[pos + i]) for i in range(g)]
            for ic in range(n_ic):
                po = p_out.tile([128, DIM], F32, tag="out", name="po")
                for i, (kt, mt) in enumerate(tiles):
                    nc.tensor.matmul(
                        po[:],
                        lhsT=kt[:, ic * 128:(ic + 1) * 128],
                        rhs=mt[:],
                        start=(i == 0), stop=(i == g - 1))
                asl = slice(ic * DIM, (ic + 1) * DIM)
                nc.vector.tensor_tensor(acc[:, asl], acc[:, asl], po[:],
                                        op=ALU.add)
            pos += g

        # ---- epilogue ----
        dma(srow_out[:], sacc[:])
        for ic in range(n_ic):
            dma(num_out[ic * 128:(ic + 1) * 128, :],
                acc[:, ic * DIM:(ic + 1) * DIM])

    nc.compile()
    return nc


# ----------------------------------------------------------------------------
# host-side preprocessing
# ----------------------------------------------------------------------------
def _kdsort(coords, nblocks):
    def rec(idx, nb):
        if nb == 1:
            return [idx]
        pts = coords[idx]
        ax = int(np.argmax(pts.max(0) - pts.min(0)))
        order = np.argsort(pts[:, ax], kind="stable")
        half = len(idx) // 2
        return rec(idx[order[:half]], nb // 2) + rec(idx[order[half:]], nb // 2)

    return np.concatenate(rec(np.arange(coords.shape[0]), nblocks))


def _split16(x):
    hi = x.astype(np.float16).astype(np.float32)
    lo = (x - hi).astype(np.float16).astype(np.float32)
    return hi, lo


def kernel(latent, coords, alpha, W1, b1, W2, b2):
    latent = np.asarray(latent, np.float32)
    coords = np.asarray(coords, np.float32)
    alpha = np.asarray(alpha, np.float32)
    W1 = np.asarray(W1, np.float32)
    b1 = np.asarray(b1, np.float32)
    W2 = np.asarray(W2, np.float32)
    b2 = np.asarray(b2, np.float32)

    perm = _kdsort(coords.astype(np.float64), 64)
    cs = coords[perm]
    als = alpha[perm]
    lats = latent[perm]
    c64 = cs.astype(np.float64)

    core_of = np.arange(N) // NSH
    # stragglers: ANY close pair (d2 < TSTRAG, i != j), grouped by i's core
    Jstar = [set() for _ in range(NCORE)]
    Istar = [set() for _ in range(NCORE)]
    d2min = np.empty(N)
    for i0 in range(0, N, 1024):
        blk = cs[i0:i0 + 1024].astype(np.float64)
        d2b = ((blk[:, None, :] - c64[None, :, :]) ** 2).sum(-1)
        d2b[np.arange(1024), np.arange(i0, i0 + 1024)] = np.inf
        d2min[i0:i0 + 1024] = d2b.min(1)
        ii, jj = np.nonzero(d2b < TSTRAG)
        ii = ii + i0
        for a, b in zip(ii, jj):
            c = core_of[a]
            Jstar[c].add(int(b))
            Istar[c].add(int(a - c * NSH))

    r = (c64 ** 2).sum(-1).astype(np.float32)
    a2 = (-2.0 * cs).astype(np.float32)
    chj = [_split16(cs[:, d]) for d in range(3)]
    ahi = [_split16(a2[:, d]) for d in range(3)]
    rj = _split16(r)
    ah = (als / 2.0).astype(np.float32)

    in_maps = []
    for core in range(NCORE):
        blk = slice(core * NSH, (core + 1) * NSH)
        rot = (np.arange(N) + core * NSH) % N   # slot row -> global row
        rows_j, rows_i = [], []
        for d in range(3):
            for (jp, ip) in [(chj[d][0], ahi[d][0]), (chj[d][0], ahi[d][1]),
                             (chj[d][1], ahi[d][0]), (chj[d][1], ahi[d][1])]:
                rows_j.append(jp)
                rows_i.append(ip[blk])
        ones = np.ones(N, np.float32)
        onesi = np.ones(NSH, np.float32)
        rows_j += [rj[0], rj[1]]
        rows_i += [onesi, onesi]
        rows_j += [ones, ones]
        rows_i += [rj[0][blk], rj[1][blk]]
        g = np.zeros(N, np.float32)
        h = np.zeros(NSH, np.float32)
        for j in Jstar[core]:
            g[j] = SUP
        for i in Istar[core]:
            h[i] = SUP
        rows_j += [g]
        rows_i += [h]
        featj = np.stack(rows_j)[:, rot].astype(np.float16)
        feati = np.stack(rows_i).astype(np.float16)

        ahj = ah[rot].reshape(NT, 128).T.copy()              # [128, 64]
        ahibc = np.broadcast_to(ah[blk], (128, NSH)).copy()
        # indirect-gather indices into the half all-gather buffers
        idxt = np.zeros((128, NT), np.int32)
        p = np.arange(128)
        for s in range(8, NT):
            gblk = (core * 8 + s) % NT
            b_ = gblk // 8
            q = (gblk % 8) * 128
            if s % 8 < 4:
                idxt[:, s] = b_ * 512 + q + p
            else:
                idxt[:, s] = b_ * 512 + (q - 512) + p
        in_maps.append({
            "featj": featj, "feati": feati,
            "ahj": np.ascontiguousarray(ahj),
            "ahibc": ahibc, "idxt": idxt,
            "latT": lats[blk].T.astype(np.float16).copy(),
            "w1t": W1.T.astype(np.float16).copy(),
            "w2t": W2.T.astype(np.float16).copy(),
            "b1c": b1.reshape(4, 128).T.astype(np.float32).copy(),
            "b2r": b2.reshape(1, DIM).astype(np.float16),
            "onescol": np.ones((1, 128), np.float16),
        })

    if "nc" not in _BUILT:
        _BUILT["nc"] = build_program()
    nc = _BUILT["nc"]
    res = run_bass_kernel_spmd(nc, in_maps, core_ids=list(range(NCORE)))

    num_all = np.zeros((N, DIM), np.float32)
    s_all = np.zeros(N, np.float32)
    msg_dev = np.zeros((N, DIM), np.float32)
    for core in range(NCORE):
        blk = slice(core * NSH, (core + 1) * NSH)
        num_all[blk] = res.results[core]["num"]
        msg_dev[blk] = res.results[core]["msgo"].astype(np.float32)
        rot = (np.arange(N) + core * NSH) % N
        s_all[rot] += res.results[core]["srow"].T.reshape(-1)

    # host fix: exact K over the suppressed straggler grid (diag excluded)
    for core in range(NCORE):
        J = sorted(Jstar[core])
        I = sorted(Istar[core])
        if not J or not I:
            continue
        Ig = np.array(I) + core * NSH
        d2c = ((c64[J][:, None, :] - c64[Ig][None, :, :]) ** 2).sum(-1)
        diag = (np.array(J)[:, None] == Ig[None, :])
        Dc = np.sqrt(d2c)
        aijc = (als[J].astype(np.float64)[:, None]
                + als[Ig].astype(np.float64)[None, :]) * 0.5
        Kc = (Dc + EPS) ** (-aijc) * np.exp(-Dc / LAM)
        Kc[diag] = 0.0
        Kc = Kc.astype(np.float32)
        num_all[Ig] += (Kc.T @ msg_dev[J]).astype(np.float32)
        s_all[np.array(J)] += Kc.sum(1)

    out = lats + num_all / (s_all[:, None] + 1e-8)
    final = np.empty_like(out)
    final[perm] = out
    return final.astype(np.float32)


# revision 14
# speedup vs baseline: 1.5504x; 1.1466x over previous
"""Trainium2 Bass kernel for ContactDiffusion GNN message passing.

out = latent + K_norm @ msg,  K = (D+eps)^(-alpha_ij) * exp(-D/12), row-normalized,
msg = MLP(latent).

v2 design (8 NeuronCores, SPMD, full inputs in / full output out):
 - Host: KD-sort points spatially; each core owns 1024 contiguous sorted rows.
 - Per-core j-block order is ROTATED by the core id so every core's own block
   sits at slots 0..7 of the 64-slot slab loop -> identical SPMD program.
 - Device: one unified Gram pass for all 64 j-tiles ([128 j x 1024 i] via a
   17-feature fp16-split matmul), elementwise chain Ln -> Exp(D/6) on ScalarE
   (single activation table set, patched chooser), (ah_i+ah_j)*l and +D/6 on
   DVE, final Exp -> bf16 K with free row-sum accumulation (accum_out).
 - Diagonal of own slots: Ln bias keeps d2>0; affine_select zeroes the diag;
   row sums for those slots via DVE reduce post-select.
 - Close pairs (d2 < TSTRAG, i != j, any core) suppressed on device via a
   rank-1 indicator feature; exact K added back on host from the device's own
   bf16 msg output. Row sums assembled on host by symmetry (column partials).
 - MLP sharded; msg AllGathered in two halves so contraction of half the
   slots can start ~30us earlier; per-slot msg tiles fetched with indirect
   DMA driven by a per-core index table (realizes the rotation).
"""

import math
import sys
from contextlib import ExitStack

import numpy as np

sys.path.insert(0, "/opt/trn_rl_repo")

import ml_dtypes

import concourse.bass as bass
import concourse.tile as tile
from concourse import bacc, mybir
from concourse.bass_utils import run_bass_kernel_spmd

F32 = mybir.dt.float32
F16 = mybir.dt.float16
BF16 = mybir.dt.bfloat16
I32 = mybir.dt.int32
AF = mybir.ActivationFunctionType
ALU = mybir.AluOpType

NP_BF16 = ml_dtypes.bfloat16

N, DIM, NCORE = 8192, 512, 8
NSH = N // NCORE
NT = N // 128               # 64 j-slots
EPS, LAM = 1e-4, 12.0
TSTRAG = 0.09
SUP = 1e3
B_LN = 2e-3                 # Ln input bias: keeps diag d2 > 0 (gram err <1.2e-3)
GROUP = 8
LN6 = math.log(6.0)

_BUILT = {}


def _patch_act_tables():
    """Force the activation-table chooser to keep Ln/Exp/Square/Copy in the
    combined natural_log_exp set (and Gelu alone in its set) so the slab loop
    runs with zero table reloads. Only narrows choices; emitted set ids still
    index the true act_info.json order."""
    import concourse.hw_specs as hw_specs

    if getattr(hw_specs.get_activation_tables, "_patched_v2", False):
        return
    orig = hw_specs.get_activation_tables
    contested = {AF.Ln, AF.Exp, AF.Square, AF.Copy, AF.Identity, AF.Gelu,
                 AF.MemsetZero}

    def patched(arch):
        tabs = orig(arch)
        out = {}
        for name, s in tabs.items():
            if name == "natural_log_exp_and_others":
                out[name] = set(s) | {AF.Copy, AF.Identity}
            elif name == "gelu_and_others":
                out[name] = (set(s) - contested) | {AF.Gelu}
            else:
                out[name] = set(s) - contested
        return out

    patched._patched_v2 = True
    hw_specs.get_activation_tables = patched
    bacc.get_activation_tables = patched


# ----------------------------------------------------------------------------
# device program
# ----------------------------------------------------------------------------
def build_program(trace_sim=False):
    nsh = NSH
    n_kd = DIM // 128           # 4 contraction k-blocks for MLP
    n_ic = nsh // 128           # 8 i-chunks

    _patch_act_tables()
    nc = bacc.Bacc("TRN2", target_bir_lowering=False, debug=False,
                   num_devices=NCORE)

    featj = nc.dram_tensor("featj", [17, N], F16, kind="ExternalInput").ap()
    feati = nc.dram_tensor("feati", [17, nsh], F16, kind="ExternalInput").ap()
    ahj = nc.dram_tensor("ahj", [128, NT], F32, kind="ExternalInput").ap()
    ahibc = nc.dram_tensor("ahibc", [128, nsh], F16, kind="ExternalInput").ap()
    idxt = nc.dram_tensor("idxt", [128, NT], I32, kind="ExternalInput").ap()
    latT = nc.dram_tensor("latT", [DIM, nsh], F16, kind="ExternalInput").ap()
    w1t = nc.dram_tensor("w1t", [DIM, DIM], F16, kind="ExternalInput").ap()
    w2t = nc.dram_tensor("w2t", [DIM, DIM], F16, kind="ExternalInput").ap()
    b1c = nc.dram_tensor("b1c", [128, n_kd], F32, kind="ExternalInput").ap()
    b2r = nc.dram_tensor("b2r", [1, DIM], F16, kind="ExternalInput").ap()
    onescol = nc.dram_tensor("onescol", [1, 128], F16, kind="ExternalInput").ap()

    num_out = nc.dram_tensor("num", [nsh, DIM], F32, kind="ExternalOutput").ap()
    srow_out = nc.dram_tensor("srow", [128, NT], F32, kind="ExternalOutput").ap()
    msgo_out = nc.dram_tensor("msgo", [nsh, DIM], BF16, kind="ExternalOutput").ap()

    with tile.TileContext(nc, trace_sim=trace_sim) as tc, ExitStack() as ctx:
        pers = ctx.enter_context(tc.tile_pool(name="pers", bufs=1))
        p_big = ctx.enter_context(tc.tile_pool(name="pbig", bufs=2, space="PSUM"))
        p_out = ctx.enter_context(tc.tile_pool(name="pout", bufs=2, space="PSUM"))
        l_pool = ctx.enter_context(tc.tile_pool(name="lp", bufs=3))
        d12_pool = ctx.enter_context(tc.tile_pool(name="d12", bufs=3))
        m_pool = ctx.enter_context(tc.tile_pool(name="mp", bufs=2))
        t_pool = ctx.enter_context(tc.tile_pool(name="tp", bufs=2))
        k_pool = ctx.enter_context(tc.tile_pool(name="kp", bufs=32))
        kraw_pool = ctx.enter_context(tc.tile_pool(name="kraw", bufs=2))
        msg_pool = ctx.enter_context(tc.tile_pool(name="msgp", bufs=26))
        dram = ctx.enter_context(tc.tile_pool(name="dram", bufs=1, space="DRAM"))

        dma = nc.sync.dma_start

        # ---- warmup collective: absorb NEFF launch skew during load phase ----
        warm_sb = pers.tile([8, 8], F32)
        nc.gpsimd.memset(warm_sb[:], 1.0)
        warm_d = dram.tile([8, 8], F32)
        warm_out = dram.tile([64, 8], F32)
        dma(warm_d[:], warm_sb[:])
        nc.gpsimd.collective_compute(
            "AllGather", ALU.bypass,
            ins=[warm_d.opt()], outs=[warm_out.opt()],
            replica_groups=[list(range(NCORE))])

        # ---- persistent SBUF loads ----
        featj_sb = pers.tile([17, N], F16)
        dma(featj_sb[:], featj[:])
        feati_sb = pers.tile([17, nsh], F16)
        dma(feati_sb[:], feati[:])
        ahj_sb = pers.tile([128, NT], F32)
        dma(ahj_sb[:], ahj[:])
        ahibc_sb = pers.tile([128, nsh], F16)
        dma(ahibc_sb[:], ahibc[:])
        idx_sb = pers.tile([128, NT], I32)
        dma(idx_sb[:], idxt[:])
        b1c_sb = pers.tile([128, n_kd], F32)
        dma(b1c_sb[:], b1c[:])
        b2r_sb = pers.tile([1, DIM], F16)
        dma(b2r_sb[:], b2r[:])
        onescol_sb = pers.tile([1, 128], F16)
        dma(onescol_sb[:], onescol[:])
        latT_sb = [pers.tile([128, nsh], F16, tag=f"latT{k}", name=f"latT{k}") for k in range(n_kd)]
        for k in range(n_kd):
            dma(latT_sb[k][:], latT[k * 128:(k + 1) * 128, :])
        w1t_sb = [pers.tile([128, DIM], F16, tag=f"w1t{k}", name=f"w1t{k}") for k in range(n_kd)]
        w2t_sb = [pers.tile([128, DIM], F16, tag=f"w2t{k}", name=f"w2t{k}") for k in range(n_kd)]
        for k in range(n_kd):
            dma(w1t_sb[k][:], w1t[k * 128:(k + 1) * 128, :])
            dma(w2t_sb[k][:], w2t[k * 128:(k + 1) * 128, :])

        acc = pers.tile([128, n_ic * DIM], F32)
        nc.gpsimd.memset(acc[:], 0.0)
        sacc = pers.tile([128, NT], F32)

        bias_b = pers.tile([128, 1], F32)
        nc.gpsimd.memset(bias_b[:], B_LN)
        bias_ln6 = pers.tile([128, 1], F32)
        nc.gpsimd.memset(bias_ln6[:], -LN6)

        msgown_d = dram.tile([nsh, DIM], BF16)
        msgall1_d = dram.tile([N // 2, DIM], BF16)   # halves of all-gathered msg
        msgall2_d = dram.tile([N // 2, DIM], BF16)

        # ---- phase A: MLP ----
        cw = 512
        hT_sb = [pers.tile([128, nsh], F16, tag=f"hT{k}", name=f"hT{k}") for k in range(n_kd)]
        for mc in range(n_kd):
            ph = p_big.tile([128, nsh], F32, tag="big", name="ph")
            for half in range(nsh // cw):
                hs = slice(half * cw, (half + 1) * cw)
                for kb in range(n_kd):
                    nc.tensor.matmul(
                        ph[:, hs],
                        lhsT=w1t_sb[kb][:, mc * 128:(mc + 1) * 128],
                        rhs=latT_sb[kb][:, hs],
                        start=(kb == 0), stop=(kb == n_kd - 1))
            nc.scalar.activation(hT_sb[mc][:], ph[:], AF.Gelu,
                                 bias=b1c_sb[:, mc:mc + 1], scale=1.0)

        msgown_sb = [pers.tile([128, DIM], BF16, tag=f"mo{ic}", name=f"mo{ic}") for ic in range(n_ic)]

        def emit_msgown(ic):
            pm = p_out.tile([128, DIM], F32, tag="out", name="pm")
            for kb in range(n_kd):
                nc.tensor.matmul(
                    pm[:],
                    lhsT=hT_sb[kb][:, ic * 128:(ic + 1) * 128],
                    rhs=w2t_sb[kb][:],
                    start=(kb == 0), stop=False)
            nc.tensor.matmul(pm[:], lhsT=onescol_sb[:], rhs=b2r_sb[:],
                             start=False, stop=True)
            nc.vector.tensor_copy(msgown_sb[ic][:], pm[:])
            dma(msgown_d[ic * 128:(ic + 1) * 128, :], msgown_sb[ic][:])
            dma(msgo_out[ic * 128:(ic + 1) * 128, :], msgown_sb[ic][:])

        for ic in range(n_ic):
            emit_msgown(ic)
        nc.gpsimd.collective_compute(
            "AllGather", ALU.bypass,
            ins=[msgown_d[0:512, :].opt()], outs=[msgall1_d.opt()],
            replica_groups=[list(range(NCORE))])
        nc.gpsimd.collective_compute(
            "AllGather", ALU.bypass,
            ins=[msgown_d[512:1024, :].opt()], outs=[msgall2_d.opt()],
            replica_groups=[list(range(NCORE))])

        # ---- slab loop ----
        # slot order: own slots first, then AG1-gated slots (s%8<4), then rest
        order = list(range(8)) \
            + [s for s in range(8, NT) if s % 8 < 4] \
            + [s for s in range(8, NT) if s % 8 >= 4]

        def emit_elementwise(s):
            pd2 = p_big.tile([128, nsh], F32, tag="big", name="pd2")
            for half in range(nsh // cw):
                hs = slice(half * cw, (half + 1) * cw)
                nc.tensor.matmul(pd2[:, hs],
                                 lhsT=featj_sb[:, s * 128:(s + 1) * 128],
                                 rhs=feati_sb[:, hs],
                                 start=True, stop=True)
            l = l_pool.tile([128, nsh], F16)
            nc.scalar.activation(l[:], pd2[:], AF.Ln, bias=bias_b[:, 0:1])
            d12 = d12_pool.tile([128, nsh], F16)
            nc.scalar.activation(d12[:], l[:], AF.Exp, bias=bias_ln6[:, 0:1],
                                 scale=0.5)
            m = m_pool.tile([128, nsh], F16)
            nc.vector.scalar_tensor_tensor(
                m[:], ahibc_sb[:], ahj_sb[:, s:s + 1], l[:],
                op0=ALU.add, op1=ALU.mult)
            t = t_pool.tile([128, nsh], F16)
            nc.vector.tensor_tensor(t[:], m[:], d12[:], op=ALU.add)
            if s < 8:
                kraw = kraw_pool.tile([128, nsh], BF16, tag="kraw", name="kraw")
                nc.scalar.activation(kraw[:], t[:], AF.Exp, scale=-0.5)
                ktile = k_pool.tile([128, nsh], BF16)
                nc.gpsimd.affine_select(
                    ktile[:], kraw[:], pattern=[[1, nsh]],
                    compare_op=ALU.not_equal, fill=0.0,
                    base=-(s * 128), channel_multiplier=-1)
                nc.vector.tensor_reduce(
                    sacc[:, s:s + 1], ktile[:], axis=mybir.AxisListType.X,
                    op=ALU.add)
                return ktile, msgown_sb[s]
            ktile = k_pool.tile([128, nsh], BF16)
            nc.scalar.activation(ktile[:], t[:], AF.Exp, scale=-0.5,
                                 accum_out=sacc[:, s:s + 1])
            mt = msg_pool.tile([128, DIM], BF16)
            src = msgall1_d if s % 8 < 4 else msgall2_d
            nc.gpsimd.indirect_dma_start(
                out=mt[:], out_offset=None, in_=src[:],
                in_offset=bass.IndirectOffsetOnAxis(ap=idx_sb[:, s:s + 1],
                                                    axis=0))
            return ktile, mt

        def emit_contraction(tiles):
            g = len(tiles)
            for ic in range(n_ic):
                po = p_out.tile([128, DIM], F32, tag="out", name="po")
                for i, (kt, mt) in enumerate(tiles):
                    nc.tensor.matmul(
                        po[:],
                        lhsT=kt[:, ic * 128:(ic + 1) * 128],
                        rhs=mt[:],
                        start=(i == 0), stop=(i == g - 1))
                asl = slice(ic * DIM, (ic + 1) * DIM)
                nc.vector.tensor_tensor(acc[:, asl], acc[:, asl], po[:],
                                        op=ALU.add)

        # software pipeline: contraction of group k runs while group k+2's
        # elementwise streams, so PE never head-of-line-blocks the Grams
        groups = [order[p:p + GROUP] for p in range(0, NT, GROUP)]
        done = []
        for gi, grp in enumerate(groups):
            done.append([emit_elementwise(s) for s in grp])
            if gi >= 2:
                emit_contraction(done[gi - 2])
        emit_contraction(done[-2])
        emit_contraction(done[-1])

        # ---- epilogue ----
        dma(srow_out[:], sacc[:])
        for ic in range(n_ic):
            dma(num_out[ic * 128:(ic + 1) * 128, :],
                acc[:, ic * DIM:(ic + 1) * DIM])

    nc.compile()
    return nc


# ----------------------------------------------------------------------------
# host-side preprocessing
# ----------------------------------------------------------------------------
def _kdsort(coords, nblocks):
    def rec(idx, nb):
        if nb == 1:
            return [idx]
        pts = coords[idx]
        ax = int(np.argmax(pts.max(0) - pts.min(0)))
        order = np.argsort(pts[:, ax], kind="stable")
        half = len(idx) // 2
        return rec(idx[order[:half]], nb // 2) + rec(idx[order[half:]], nb // 2)

    return np.concatenate(rec(np.arange(coords.shape[0]), nblocks))


def _split16(x):
    hi = x.astype(np.float16).astype(np.float32)
    lo = (x - hi).astype(np.float16).astype(np.float32)
    return hi, lo


def kernel(latent, coords, alpha, W1, b1, W2, b2):
    latent = np.asarray(latent, np.float32)
    coords = np.asarray(coords, np.float32)
    alpha = np.asarray(alpha, np.float32)
    W1 = np.asarray(W1, np.float32)
    b1 = np.asarray(b1, np.float32)
    W2 = np.asarray(W2, np.float32)
    b2 = np.asarray(b2, np.float32)

    perm = _kdsort(coords.astype(np.float64), 64)
    cs = coords[perm]
    als = alpha[perm]
    lats = latent[perm]
    c64 = cs.astype(np.float64)

    core_of = np.arange(N) // NSH
    # stragglers: ANY close pair (d2 < TSTRAG, i != j), grouped by i's core
    Jstar = [set() for _ in range(NCORE)]
    Istar = [set() for _ in range(NCORE)]
    d2min = np.empty(N)
    for i0 in range(0, N, 1024):
        blk = cs[i0:i0 + 1024].astype(np.float64)
        d2b = ((blk[:, None, :] - c64[None, :, :]) ** 2).sum(-1)
        d2b[np.arange(1024), np.arange(i0, i0 + 1024)] = np.inf
        d2min[i0:i0 + 1024] = d2b.min(1)
        ii, jj = np.nonzero(d2b < TSTRAG)
        ii = ii + i0
        for a, b in zip(ii, jj):
            c = core_of[a]
            Jstar[c].add(int(b))
            Istar[c].add(int(a - c * NSH))

    r = (c64 ** 2).sum(-1).astype(np.float32)
    a2 = (-2.0 * cs).astype(np.float32)
    chj = [_split16(cs[:, d]) for d in range(3)]
    ahi = [_split16(a2[:, d]) for d in range(3)]
    rj = _split16(r)
    ah = (als / 2.0).astype(np.float32)

    in_maps = []
    for core in range(NCORE):
        blk = slice(core * NSH, (core + 1) * NSH)
        rot = (np.arange(N) + core * NSH) % N   # slot row -> global row
        rows_j, rows_i = [], []
        for d in range(3):
            for (jp, ip) in [(chj[d][0], ahi[d][0]), (chj[d][0], ahi[d][1]),
                             (chj[d][1], ahi[d][0]), (chj[d][1], ahi[d][1])]:
                rows_j.append(jp)
                rows_i.append(ip[blk])
        ones = np.ones(N, np.float32)
        onesi = np.ones(NSH, np.float32)
        rows_j += [rj[0], rj[1]]
        rows_i += [onesi, onesi]
        rows_j += [ones, ones]
        rows_i += [rj[0][blk], rj[1][blk]]
        g = np.zeros(N, np.float32)
        h = np.zeros(NSH, np.float32)
        for j in Jstar[core]:
            g[j] = SUP
        for i in Istar[core]:
            h[i] = SUP
        rows_j += [g]
        rows_i += [h]
        featj = np.stack(rows_j)[:, rot].astype(np.float16)
        feati = np.stack(rows_i).astype(np.float16)

        ahj = ah[rot].reshape(NT, 128).T.copy()              # [128, 64]
        ahibc = np.broadcast_to(ah[blk], (128, NSH)).astype(np.float16).copy()
        # indirect-gather indices into the half all-gather buffers
        idxt = np.zeros((128, NT), np.int32)
        p = np.arange(128)
        for s in range(8, NT):
            gblk = (core * 8 + s) % NT
            b_ = gblk // 8
            q = (gblk % 8) * 128
            if s % 8 < 4:
                idxt[:, s] = b_ * 512 + q + p
            else:
                idxt[:, s] = b_ * 512 + (q - 512) + p
        in_maps.append({
            "featj": featj, "feati": feati,
            "ahj": np.ascontiguousarray(ahj),
            "ahibc": ahibc, "idxt": idxt,
            "latT": lats[blk].T.astype(np.float16).copy(),
            "w1t": W1.T.astype(np.float16).copy(),
            "w2t": W2.T.astype(np.float16).copy(),
            "b1c": b1.reshape(4, 128).T.astype(np.float32).copy(),
            "b2r": b2.reshape(1, DIM).astype(np.float16),
            "onescol": np.ones((1, 128), np.float16),
        })

    if "nc" not in _BUILT:
        _BUILT["nc"] = build_program()
    nc = _BUILT["nc"]
    res = run_bass_kernel_spmd(nc, in_maps, core_ids=list(range(NCORE)))

    num_all = np.zeros((N, DIM), np.float32)
    s_all = np.zeros(N, np.float32)
    msg_dev = np.zeros((N, DIM), np.float32)
    for core in range(NCORE):
        blk = slice(core * NSH, (core + 1) * NSH)
        num_all[blk] = res.results[core]["num"]
        msg_dev[blk] = res.results[core]["msgo"].astype(np.float32)
        rot = (np.arange(N) + core * NSH) % N
        s_all[rot] += res.results[core]["srow"].T.reshape(-1)

    # host fix: exact K over the suppressed straggler grid (diag excluded)
    for core in range(NCORE):
        J = sorted(Jstar[core])
        I = sorted(Istar[core])
        if not J or not I:
            continue
        Ig = np.array(I) + core * NSH
        d2c = ((c64[J][:, None, :] - c64[Ig][None, :, :]) ** 2).sum(-1)
        diag = (np.array(J)[:, None] == Ig[None, :])
        Dc = np.sqrt(d2c)
        aijc = (als[J].astype(np.float64)[:, None]
                + als[Ig].astype(np.float64)[None, :]) * 0.5
        Kc = (Dc + EPS) ** (-aijc) * np.exp(-Dc / LAM)
        Kc[diag] = 0.0
        Kc = Kc.astype(np.float32)
        num_all[Ig] += (Kc.T @ msg_dev[J]).astype(np.float32)
        s_all[np.array(J)] += Kc.sum(1)

    out = lats + num_all / (s_all[:, None] + 1e-8)
    final = np.empty_like(out)
    final[perm] = out
    return final.astype(np.float32)
